# revision 23
# baseline (speedup 1.0000x reference)
"""Trainium2 Bass kernel for nn_DistributionSimilarity.

Per query q (8 queries, one per NeuronCore):
    ed[j,z]    = mean_k exp(-(v[j,k]-v[z,k])^2)          (j,z < 1024, k < 64)
    later[j,z] = softmax(ed, axis=-1)[j,z] * (1 - eye)[j,z]

Method: the Gaussian kernel is separable via a cosine quadrature,
    exp(-d^2) ~= w0 + sum_{m=1..NM} w_m cos(m*DT*d)
and cos(t(x-y)) = cos(tx)cos(ty) + sin(tx)sin(ty), so with features
F_m = sqrt(w_m/64)*[cos(t_m x); sin(t_m x)] (K=128 = 64 support x {cos,sin}):
    ed = w0 + sum_m F_m^T F_m  -- pure TensorE Gram matmuls; the w0 constant
rides along for free as an activation bias in the epilogue.

TensorE runs fp32 at 1/4 rate, so each F_m is split fp16 hi/lo (H + L) and
ed accumulates H^T H + H^T L + L^T H in PSUM (the L^T L term is ~1e-7; cross
terms are dropped for small-weight nodes m > CROSS_MAX). ScalarE Sin (valid
only on [-pi,pi]) gets range-reduced input via a DVE magic-number round.

ed is symmetric: rows j>=512 are computed fully ("wave A"); for rows j<512
only the left half is computed by matmul and the upper-right quarter is
reconstructed with PE transposes of wave-A results into the same PSUM banks
("wave B"). Softmax reads PSUM directly; no max-subtraction is needed since
ed is in (0, 1].

Sharding: data-parallel over n_query; core q handles query q. No collectives.
"""
import math
from contextlib import ExitStack

import numpy as np

import concourse.bacc as bacc
import concourse.bass as bass
import concourse.tile as tile
from concourse import mybir
from concourse.bass_utils import run_bass_kernel_spmd

F32 = mybir.dt.float32
F16 = mybir.dt.float16
AF = mybir.ActivationFunctionType
ALU = mybir.AluOpType

N_QUERY, N_SAMPLE, N_SUPPORT = 8, 1024, 64
N_CORES = 8

# Quadrature: trapezoid on the Gaussian's Fourier transform; max err ~3.3e-6
# over |d| <= 10.1 (data range is |d| < 10.03) including the fp16 split.
NM = 14
DT = 0.46
CROSS_MAX = 6  # fp16 hi/lo cross-correction matmuls only for m <= CROSS_MAX
MAGIC = 1.5 * 2.0**23  # fp32 round-to-nearest-int magic constant
TWO_PI = 2.0 * math.pi

_W = [DT / math.sqrt(math.pi) * math.exp(-((m * DT) ** 2) / 4.0) for m in range(NM + 1)]
_W[0] *= 0.5  # trapezoid half-weight at t=0
_SW = [math.sqrt(w / N_SUPPORT) for w in _W]  # symmetric sqrt-weights
W0 = _W[0]
# node-0 constant feature, fp16 hi/lo split; the tiny remainder rides as an
# epilogue bias. h0/l0 matmul schemes double as PE warm-up during the
# feature-pipeline lead-in.
H0V = float(np.float16(_SW[0]))
L0V = float(np.float16(_SW[0] - H0V))
W0_RES = W0 - N_SUPPORT * (H0V * H0V + 2.0 * H0V * L0V)

_COMPILED = None


def _build():
    nc = bacc.Bacc("TRN2", target_bir_lowering=False, debug=False)

    # x2: [vT; vT] pre-stacked on host. constf: mask(0:128) | ident(128:256) |
    # qcol(256) | w0col(257).
    x2_d = nc.declare_dram_parameter("x2", [128, N_SAMPLE], F32, isOutput=False)
    cf_d = nc.declare_dram_parameter("constf", [128, 259], F32, isOutput=False)
    ed_d = nc.declare_dram_parameter("ed", [N_SAMPLE, N_SAMPLE], F32, isOutput=True)
    later_d = nc.declare_dram_parameter("later", [N_SAMPLE, N_SAMPLE], F32, isOutput=True)

    with tile.TileContext(nc) as tc, ExitStack() as ctx:
        singles = ctx.enter_context(tc.tile_pool(name="singles", bufs=1))
        feats = ctx.enter_context(tc.tile_pool(name="feats", bufs=1))
        temps = ctx.enter_context(tc.tile_pool(name="temps", bufs=2))
        stage = ctx.enter_context(tc.tile_pool(name="stage", bufs=3))
        psum = ctx.enter_context(tc.tile_pool(name="psum", bufs=4, space="PSUM"))

        # --- input staging --------------------------------------------------
        cf = singles.tile([128, 259], F32)
        nc.gpsimd.dma_start(out=cf, in_=cf_d[:, :])
        x2 = singles.tile([128, N_SAMPLE], F32)
        nc.gpsimd.dma_start(out=x2, in_=x2_d[:, :])
        maskb = cf[:, 0:128]
        ident = cf[:, 128:256]
        qcol = cf[:, 256:257]
        w0col = cf[:, 257:258]
        w0bcol = cf[:, 258:259]

        h0t = feats.tile([128, N_SAMPLE], F16, tag="H0", name="H0")
        h_t = [None] + [
            feats.tile([128, N_SAMPLE], F16, tag=f"H{m}", name=f"H{m}")
            for m in range(1, NM + 1)
        ]
        nc.vector.memset(h0t[0:64, :], H0V)
        nc.vector.memset(h0t[64:128, :], 0.0)
        l0t = feats.tile([128, N_SAMPLE], F16, tag="L0", name="L0")
        nc.vector.memset(l0t[0:64, :], L0V)
        nc.vector.memset(l0t[64:128, :], 0.0)
        l_t = [None] + [
            feats.tile([128, N_SAMPLE], F16, tag=f"L{m}", name=f"L{m}")
            for m in range(1, CROSS_MAX + 1)
        ]

        # --- features: H_m = fp16(sw*sin/cos), L_m = fp16(sw*f - H_m) -------
        for m in range(1, NM + 1):
            t = m * DT
            sw = _SW[m]
            prio = tc.high_priority() if m == 1 else None
            if prio is not None:
                prio.__enter__()
            y = temps.tile([128, N_SAMPLE], F32, tag="y")
            k = temps.tile([128, N_SAMPLE], F32, tag="k")
            r = temps.tile([128, N_SAMPLE], F32, tag="r")
            f = temps.tile([128, N_SAMPLE], F32, tag="f", bufs=3)
            # y = x * t/2pi + {1/4 top, 0 bottom}  (angle in turns)
            nc.vector.tensor_scalar(y, x2, t / TWO_PI, qcol, ALU.mult, ALU.add)
            # k = round(y); r = y - k in [-0.5, 0.5]
            nc.vector.tensor_scalar(k, y, MAGIC, MAGIC, ALU.add, ALU.subtract)
            nc.vector.scalar_tensor_tensor(r, y, 1.0, k, ALU.mult, ALU.subtract)
            nc.scalar.activation(f, r, AF.Sin, bias=0.0, scale=TWO_PI)
            nc.scalar.activation(h_t[m], f, AF.Copy, bias=0.0, scale=sw)
            if m <= CROSS_MAX:
                nc.vector.scalar_tensor_tensor(
                    l_t[m], f, sw, h_t[m], ALU.mult, ALU.subtract
                )
            if prio is not None:
                prio.__exit__(None, None, None)

        # matmul plan: cross terms for node m go right after H_m (PE gets
        # dense work while later features are computed); crosses of m=6,7 are
        # deferred into the crossless m>=9 window.
        schemes = [(h0t, h0t), (h0t, l0t), (l0t, h0t)]
        deferred = []
        for m in range(1, NM + 1):
            schemes.append((h_t[m], h_t[m]))
            if m <= CROSS_MAX:
                if m < 6:
                    schemes.append((h_t[m], l_t[m]))
                    schemes.append((l_t[m], h_t[m]))
                else:
                    deferred.append((h_t[m], l_t[m]))
                    deferred.append((l_t[m], h_t[m]))
            elif deferred:
                schemes.append(deferred.pop(0))
                schemes.append(deferred.pop(0))
        schemes.extend(deferred)
        n_sch = len(schemes)

        edt_keep = [
            singles.tile([128, N_SAMPLE], F32, name=f"edk{i}") for i in range(4)
        ]

        def softmax_tail(jt, expt, rc):
            outt = stage.tile([128, N_SAMPLE], F32, tag="outt")
            nc.vector.tensor_scalar(outt, expt, rc, None, ALU.mult)
            nc.vector.tensor_tensor(
                outt[:, jt * 128 : (jt + 1) * 128],
                outt[:, jt * 128 : (jt + 1) * 128],
                maskb,
                ALU.mult,
            )
            nc.sync.dma_start(out=later_d[jt * 128 : (jt + 1) * 128, :], in_=outt)

        def epilogue_a(jt, pt, edt, nleft):
            # exp first (ACT) and ed-copy on DVE in parallel -> the PSUM bank
            # frees as early as possible for wave B; +w0 rides as bias.
            # cols [nleft:]: transposed blocks that already include w0.
            expt = stage.tile([128, N_SAMPLE], F32, tag="expt")
            rs = stage.tile([128, 1], F32, tag="rs")
            rc = stage.tile([128, 1], F32, tag="rc")
            if nleft == 1024:
                nc.scalar.activation(
                    expt, pt, AF.Exp, bias=w0col, scale=1.0, accum_out=rs
                )
                nc.vector.tensor_scalar(edt, pt, W0_RES, None, ALU.add)
                nc.vector.reciprocal(rc, rs)
            else:
                rs1 = stage.tile([128, 1], F32, tag="rs1a")
                nc.scalar.activation(
                    expt[:, 0:nleft], pt[:, 0:nleft], AF.Exp, bias=w0col, scale=1.0,
                    accum_out=rs,
                )
                nc.scalar.activation(
                    expt[:, nleft:1024], pt[:, nleft:1024], AF.Exp, bias=0.0,
                    scale=1.0, accum_out=rs1,
                )
                nc.vector.tensor_scalar(
                    edt[:, 0:nleft], pt[:, 0:nleft], W0_RES, None, ALU.add
                )
                nc.vector.tensor_copy(edt[:, nleft:1024], pt[:, nleft:1024])
                nc.vector.tensor_tensor(rc, rs, rs1, ALU.add)
                nc.vector.reciprocal(rc, rc)
            nc.sync.dma_start(out=ed_d[jt * 128 : (jt + 1) * 128, :], in_=edt)
            softmax_tail(jt, expt, rc)

        def epilogue_b(jt, pt, edt, nleft, split_tail=False):
            # cols [0:nleft]: matmul result, needs +w0; cols [nleft:]:
            # transposed blocks, already include w0. exp first (critical path).
            expt = stage.tile([128, N_SAMPLE], F32, tag="expt")
            rs0 = stage.tile([128, 1], F32, tag="rs0")
            rs1 = stage.tile([128, 1], F32, tag="rs1")
            # ed-copy first in program order: later tiles' transposes read it
            nc.vector.tensor_scalar(
                edt[:, 0:nleft], pt[:, 0:nleft], W0, None, ALU.add
            )
            nc.scalar.activation(
                expt[:, 0:nleft], pt[:, 0:nleft], AF.Exp, bias=w0bcol, scale=1.0,
                accum_out=rs0,
            )
            nc.scalar.activation(
                expt[:, nleft:1024], pt[:, nleft:1024], AF.Exp, bias=0.0, scale=1.0,
                accum_out=rs1,
            )
            if split_tail:
                nc.sync.dma_start(
                    out=ed_d[jt * 128 : (jt + 1) * 128, 0:nleft], in_=edt[:, 0:nleft]
                )
                nc.scalar.copy(edt[:, nleft:1024], pt[:, nleft:1024])
                nc.sync.dma_start(
                    out=ed_d[jt * 128 : (jt + 1) * 128, nleft:1024],
                    in_=edt[:, nleft:1024],
                )
            else:
                nc.scalar.copy(edt[:, nleft:1024], pt[:, nleft:1024])
                nc.sync.dma_start(out=ed_d[jt * 128 : (jt + 1) * 128, :], in_=edt)
            if split_tail:
                # mask the exp tile before the denominator is ready: keeps the
                # diagonal zeroing off the final critical path (the masked
                # entries don't feed rs0/rs1 -- accumulation already ran)
                nc.vector.tensor_tensor(
                    expt[:, jt * 128 : (jt + 1) * 128],
                    expt[:, jt * 128 : (jt + 1) * 128],
                    maskb,
                    ALU.mult,
                )
            rc = stage.tile([128, 1], F32, tag="rc")
            nc.vector.tensor_tensor(rc, rs0, rs1, ALU.add)
            nc.vector.reciprocal(rc, rc)
            if not split_tail:
                softmax_tail(jt, expt, rc)
                return
            # pipelined split tail for the final tile
            outt = stage.tile([128, N_SAMPLE], F32, tag="outt")
            nc.vector.tensor_scalar(outt[:, 0:512], expt[:, 0:512], rc, None, ALU.mult)
            nc.sync.dma_start(
                out=later_d[jt * 128 : (jt + 1) * 128, 0:512], in_=outt[:, 0:512]
            )
            nc.vector.tensor_scalar(
                outt[:, 512:1024], expt[:, 512:1024], rc, None, ALU.mult
            )
            nc.sync.dma_start(
                out=later_d[jt * 128 : (jt + 1) * 128, 512:1024],
                in_=outt[:, 512:1024],
            )

        # --- wave A: j-tiles 4..7, triangular above the diagonal ------------
        # ed[jt-rows, z >= (jt+1)*128] is reconstructed by transposing blocks
        # of later wave-A tiles; matmuls span cols [0:(jt+1)*128] only.
        pa = {i: psum.tile([128, N_SAMPLE], F32, tag="ps", name=f"pa{i}") for i in range(4, 8)}
        nla = {jt: (jt + 1) * 128 for jt in range(4, 8)}
        for si, (lt, rt) in enumerate(schemes):
            for jt in (7, 6, 5, 4):
                for lo, hi in ((0, 512), (512, nla[jt])):
                    nc.tensor.matmul(
                        pa[jt][:, lo:hi],
                        lt[:, jt * 128 : (jt + 1) * 128],
                        rt[:, lo:hi],
                        start=(si == 0),
                        stop=(si == n_sch - 1),
                    )
        for jt in (7, 6, 5, 4):
            for zb in range(jt + 1, 8):
                nc.tensor.transpose(
                    pa[jt][:, zb * 128 : (zb + 1) * 128],
                    edt_keep[zb - 4][:, jt * 128 : (jt + 1) * 128],
                    ident,
                )
            epilogue_a(jt, pa[jt], edt_keep[jt - 4], nla[jt])

        # --- wave B: j-tiles 0..3, triangular, jt descending ----------------
        # ed[jt-rows, z >= (jt+1)*128] is above the diagonal: reconstructed by
        # transposing blocks from later j-tiles (wave A's edt_keep and wave
        # B's own earlier tiles), so the matmul only spans cols [0:(jt+1)*128].
        edtB = {}
        for jt in (3, 2, 1, 0):
            nleft = (jt + 1) * 128
            pbt = psum.tile([128, N_SAMPLE], F32, tag="ps", name=f"pb{jt}")
            for si, (lt, rt) in enumerate(schemes[3:]):
                nc.tensor.matmul(
                    pbt[:, 0:nleft],
                    lt[:, jt * 128 : (jt + 1) * 128],
                    rt[:, 0:nleft],
                    start=(si == 0),
                    stop=(si == n_sch - 4),
                )
            for zb in range(jt + 1, 8):
                # ed[jt-block, zb-block] = ed[zb-block, jt-block]^T
                src = edt_keep[zb - 4] if zb >= 4 else edtB[zb]
                nc.tensor.transpose(
                    pbt[:, zb * 128 : (zb + 1) * 128],
                    src[:, jt * 128 : (jt + 1) * 128],
                    ident,
                )
            if jt > 0:
                edt = edtB[jt] = singles.tile(
                    [128, N_SAMPLE], F32, name=f"edtB{jt}"
                )
            else:
                edt = stage.tile([128, N_SAMPLE], F32, tag="edt")
            epilogue_b(jt, pbt, edt, nleft, split_tail=(jt == 0))

    nc.compile()
    return nc


def _get_nc():
    global _COMPILED
    if _COMPILED is None:
        _COMPILED = _build()
    return _COMPILED


def _make_in_maps(v):
    constf = np.zeros((128, 259), np.float32)
    constf[:, 0:128] = 1.0 - np.eye(128)
    constf[:, 128:256] = np.eye(128)
    constf[0:64, 256] = 0.25
    constf[:, 257] = W0_RES
    constf[:, 258] = W0
    return [
        {
            "x2": np.ascontiguousarray(np.vstack([v[q].T, v[q].T])),
            "constf": constf,
        }
        for q in range(N_QUERY)
    ]


def kernel(vd_curr_gen, distance_metric=None, **_ignored):
    v = np.ascontiguousarray(np.asarray(vd_curr_gen, dtype=np.float32))
    assert v.shape == (N_QUERY, N_SAMPLE, N_SUPPORT), v.shape
    nc = _get_nc()
    try:
        res = run_bass_kernel_spmd(nc, _make_in_maps(v), core_ids=list(range(N_CORES)))
    except Exception:
        # transient accelerator hiccups have been observed; retry once
        import time as _time

        _time.sleep(5)
        res = run_bass_kernel_spmd(nc, _make_in_maps(v), core_ids=list(range(N_CORES)))
    ed = np.stack([res.results[q]["ed"] for q in range(N_QUERY)])
    later = np.stack([res.results[q]["later"] for q in range(N_QUERY)])
    return ed, later
